# revision 2
# baseline (speedup 1.0000x reference)
import os

import numpy as np

B = 512
H = W = 112
HW = H * W
NCORES = 8
BPC = B // NCORES
P = 128
HALF = HW // 2
QROW = HW // 4
SLOTS = 32
CAP = NCORES * SLOTS

CHUNKS_FULL = [784] * 7 + [560, 224]
assert sum(CHUNKS_FULL) == HALF
CHUNKS_MASK = [560] * 5 + [336]
assert sum(CHUNKS_MASK) == QROW

_NC_CACHE = {}


def _build_nc(masked):
    import concourse.bacc as bacc
    import concourse.tile as tile
    from concourse import mybir

    import bass_rust
    from concourse.hw_specs import get_activation_tables

    f32 = mybir.dt.float32
    AF = mybir.ActivationFunctionType
    OP = mybir.AluOpType
    AX = mybir.AxisListType

    chunks = CHUNKS_MASK if masked else CHUNKS_FULL
    row = QROW if masked else HALF

    nc = bacc.Bacc("TRN2", target_bir_lowering=False, debug=False,
                   num_devices=NCORES)
    act_set_id = list(get_activation_tables("gen3").keys()).index(
        "natural_log_exp_and_others")
    abc = nc.dram_tensor("abc", [P, 3 * row], f32, kind="ExternalInput").ap()
    small = nc.dram_tensor("small", [P, 9], f32, kind="ExternalInput").ap()
    if masked:
        small_cam = nc.dram_tensor("small_cam", [P, 9], f32,
                                   kind="ExternalInput").ap()
    outp = nc.dram_tensor("out", [1, 1], f32, kind="ExternalOutput").ap()

    with tile.TileContext(nc) as tc:
        with (
            tc.tile_pool(name="big", bufs=6) as big,
            tc.tile_pool(name="sm", bufs=1) as sm,
            tc.tile_pool(name="ps", bufs=1, space="PSUM") as ps,
        ):
            nc.scalar.add_instruction(bass_rust.InstLoadActFuncSet(
                name=nc.get_next_instruction_name(),
                engine=mybir.EngineType.Activation,
                act_func_set_id=act_set_id,
            ))

            smt = sm.tile([P, 9], f32)
            nc.gpsimd.dma_start(out=smt, in_=small)
            if masked:
                smc = sm.tile([P, 9], f32)
                nc.gpsimd.dma_start(out=smc, in_=small_cam)
            ones = sm.tile([P, 1], f32)
            nc.vector.memset(ones, 1.0)

            NCHUNK = len(chunks)
            er_parts = sm.tile([P, NCHUNK], f32)
            sp_parts = sm.tile([P, NCHUNK], f32)

            def lse2(ps_ap, tag):
                mx = sm.tile([P, 1], f32, tag=f"mx_{tag}")
                nc.vector.reduce_max(mx, ps_ap, axis=AX.X)
                dd = sm.tile([P, 1], f32, tag=f"dd_{tag}")
                nc.vector.tensor_sub(dd, ps_ap[:, 1:2], ps_ap[:, 0:1])
                nad = sm.tile([P, 1], f32, tag=f"nad_{tag}")
                nc.vector.tensor_scalar_mul(nad, dd, -1.0)
                nc.vector.tensor_tensor(out=nad, in0=dd, in1=nad, op=OP.min)
                spt = sm.tile([P, 1], f32, tag=f"sp_{tag}")
                nc.scalar.activation(out=spt, in_=nad, func=AF.Exp)
                nc.scalar.activation(out=spt, in_=spt, func=AF.Ln, bias=1.0)
                ls = sm.tile([P, 1], f32, tag=f"ls_{tag}")
                nc.vector.tensor_add(ls, mx, spt)
                return ls, dd

            def weight_chain(p1, p1o, yf, tag):
                ls1, d1 = lse2(p1, f"p1_{tag}")
                pm = sm.tile([P, 1], f32, tag=f"pm_{tag}")
                nc.vector.tensor_sub(pm, p1[:, 1:2], ls1)
                prob1 = sm.tile([P, 1], f32, tag=f"pr_{tag}")
                nc.scalar.activation(out=prob1, in_=pm, func=AF.Exp)
                cur = sm.tile([P, 1], f32, tag=f"cur_{tag}")
                nc.vector.tensor_tensor(out=cur, in0=p1[:, 1:2],
                                        in1=p1[:, 0:1], op=OP.is_gt)
                flag = sm.tile([P, 1], f32, tag=f"flag_{tag}")
                nc.vector.tensor_tensor(out=flag, in0=p1o[:, 1:2],
                                        in1=p1o[:, 0:1], op=OP.is_gt)
                neq = sm.tile([P, 1], f32, tag=f"neq_{tag}")
                nc.vector.tensor_tensor(out=neq, in0=cur, in1=flag,
                                        op=OP.not_equal)
                sameflag = sm.tile([P, 1], f32, tag=f"same_{tag}")
                nc.vector.tensor_scalar(out=sameflag, in0=neq, scalar1=-1.0,
                                        scalar2=1.0, op0=OP.mult, op1=OP.add)
                om = sm.tile([P, 1], f32, tag=f"om_{tag}")
                nc.vector.tensor_scalar(out=om, in0=cur, scalar1=-1.0,
                                        scalar2=1.0, op0=OP.mult, op1=OP.add)
                cond = sm.tile([P, 1], f32, tag=f"cond_{tag}")
                nc.vector.tensor_mul(cond, neq, om)
                nc.vector.tensor_mul(cond, cond, yf)
                p1m1 = sm.tile([P, 1], f32, tag=f"p1m1_{tag}")
                nc.vector.tensor_scalar_add(p1m1, prob1, -1.0)
                wv = sm.tile([P, 1], f32, tag=f"wv_{tag}")
                nc.vector.tensor_mul(wv, cond, p1m1)
                nc.vector.tensor_scalar_add(wv, wv, 1.0)
                return wv, sameflag, ls1, d1

            def sigmoid_weight_chain(p1, p1o, yf, tag):
                d1 = sm.tile([P, 1], f32, tag=f"d1_{tag}")
                nc.vector.tensor_sub(d1, p1[:, 1:2], p1[:, 0:1])
                nd = sm.tile([P, 1], f32, tag=f"nd_{tag}")
                nc.vector.tensor_scalar_mul(nd, d1, -1.0)
                prob1 = sm.tile([P, 1], f32, tag=f"pr_{tag}")
                nc.scalar.activation(out=prob1, in_=nd, func=AF.Exp)
                nc.vector.tensor_scalar_add(prob1, prob1, 1.0)
                nc.vector.reciprocal(prob1, prob1)
                cur = sm.tile([P, 1], f32, tag=f"cur_{tag}")
                nc.vector.tensor_tensor(out=cur, in0=p1[:, 1:2],
                                        in1=p1[:, 0:1], op=OP.is_gt)
                flag = sm.tile([P, 1], f32, tag=f"flag_{tag}")
                nc.vector.tensor_tensor(out=flag, in0=p1o[:, 1:2],
                                        in1=p1o[:, 0:1], op=OP.is_gt)
                neq = sm.tile([P, 1], f32, tag=f"neq_{tag}")
                nc.vector.tensor_tensor(out=neq, in0=cur, in1=flag,
                                        op=OP.not_equal)
                sameflag = sm.tile([P, 1], f32, tag=f"same_{tag}")
                nc.vector.tensor_scalar(out=sameflag, in0=neq, scalar1=-1.0,
                                        scalar2=1.0, op0=OP.mult, op1=OP.add)
                om = sm.tile([P, 1], f32, tag=f"om_{tag}")
                nc.vector.tensor_scalar(out=om, in0=cur, scalar1=-1.0,
                                        scalar2=1.0, op0=OP.mult, op1=OP.add)
                cond = sm.tile([P, 1], f32, tag=f"cond_{tag}")
                nc.vector.tensor_mul(cond, neq, om)
                nc.vector.tensor_mul(cond, cond, yf)
                p1m1 = sm.tile([P, 1], f32, tag=f"p1m1_{tag}")
                nc.vector.tensor_scalar_add(p1m1, prob1, -1.0)
                wv = sm.tile([P, 1], f32, tag=f"wv_{tag}")
                nc.vector.tensor_mul(wv, cond, p1m1)
                nc.vector.tensor_scalar_add(wv, wv, 1.0)
                return wv, sameflag

            if masked:
                yfc = smc[:, 8:9]
                wc, samec = sigmoid_weight_chain(smc[:, 0:2], smc[:, 2:4],
                                                 yfc, "cam")
            else:
                yfc = smt[:, 8:9]
                wc, samec = sigmoid_weight_chain(smt[:, 0:2], smt[:, 2:4],
                                                 yfc, "camf")
            coef_er = sm.tile([P, 1], f32)
            nc.vector.scalar_tensor_tensor(out=coef_er, in0=wc,
                                           scalar=1.0 / (B * HW), in1=yfc,
                                           op0=OP.mult, op1=OP.mult)
            coef_sp = sm.tile([P, 1], f32)
            nc.vector.scalar_tensor_tensor(out=coef_sp, in0=samec,
                                           scalar=1.0 / (B * HW), in1=yfc,
                                           op0=OP.mult, op1=OP.mult)

            cepart = sm.tile([P, 1], f32)

            def ce_chain():
                p1 = smt[:, 0:2]
                p2 = smt[:, 4:6]
                pb = smt[:, 6:8]
                yf = smt[:, 8:9]
                wv, _, ls1, d1 = weight_chain(p1, smt[:, 2:4], yf, "ce")
                yield
                ls2_, d2 = lse2(p2, "p2")
                yield
                lsb, _ = lse2(pb, "pb")
                yield
                sel1 = sm.tile([P, 1], f32)
                nc.vector.tensor_mul(sel1, yf, d1)
                nc.vector.tensor_add(sel1, p1[:, 0:1], sel1)
                ce1 = sm.tile([P, 1], f32)
                nc.vector.tensor_sub(ce1, ls1, sel1)
                yield
                sel2 = sm.tile([P, 1], f32)
                nc.vector.tensor_mul(sel2, yf, d2)
                nc.vector.tensor_add(sel2, p2[:, 0:1], sel2)
                ce2 = sm.tile([P, 1], f32)
                nc.vector.tensor_sub(ce2, ls2_, sel2)
                yield
                q = sm.tile([P, 1], f32)
                nc.vector.tensor_add(q, ce1, ce2)
                cebr = sm.tile([P, 1], f32)
                nc.vector.tensor_sub(cebr, lsb, pb[:, 0:1])
                nc.vector.tensor_mul(cebr, cebr, yf)
                nc.vector.tensor_add(q, q, cebr)
                yield
                nc.vector.scalar_tensor_tensor(out=cepart, in0=q,
                                               scalar=1.0 / (4 * B), in1=wv,
                                               op0=OP.mult, op1=OP.mult)

            ce_steps = ce_chain()
            pt = ps.tile([1, 1], f32)

            off = 0
            for ci, cf in enumerate(chunks):
                last = ci == len(chunks) - 1
                abct = big.tile([P, 3 * cf], f32, tag="abct")
                nc.sync.dma_start(out=abct, in_=abc[:, 3 * off:3 * (off + cf)])
                off += cf
                at = abct[:, 0:cf]
                bt = abct[:, cf:2 * cf]
                ct = abct[:, 2 * cf:3 * cf]
                d = big.tile([P, cf], f32, tag="d")
                nc.vector.tensor_sub(d, at, bt)
                if last:
                    nc.vector.affine_mul_reduce(
                        out=d, accum_out=er_parts[:, ci:ci + 1],
                        in0=d, in1=d, scale=1.0, bias=0.0)
                else:
                    nc.scalar.activation(out=d, in_=d, func=AF.Square,
                                         accum_out=er_parts[:, ci:ci + 1])
                nc.tensor.matmul(out=pt, lhsT=coef_er,
                                 rhs=er_parts[:, ci:ci + 1], start=(ci == 0),
                                 stop=False)
                e = big.tile([P, cf], f32, tag="e")
                nc.vector.tensor_sub(e, at, ct)
                if last:
                    nc.vector.affine_mul_reduce(
                        out=e, accum_out=sp_parts[:, ci:ci + 1],
                        in0=e, in1=e, scale=1.0, bias=0.0)
                else:
                    nc.scalar.activation(out=e, in_=e, func=AF.Square,
                                         accum_out=sp_parts[:, ci:ci + 1])
                nc.tensor.matmul(out=pt, lhsT=coef_sp,
                                 rhs=sp_parts[:, ci:ci + 1], start=False,
                                 stop=False)
                next(ce_steps, None)

            for _ in ce_steps:
                pass
            nc.tensor.matmul(out=pt, lhsT=cepart, rhs=ones, start=False,
                             stop=True)

            res_sb = sm.tile([1, 1], f32)
            nc.vector.tensor_copy(res_sb, pt)
            nc.sync.dma_start(out=outp, in_=res_sb)

    nc.compile()
    return nc


def _get_nc(masked):
    key = "mask" if masked else "full"
    if key not in _NC_CACHE:
        _NC_CACHE[key] = _build_nc(masked)
    return _NC_CACHE[key]


def _interleave(a, b, c, chunks):
    row = a.shape[1]
    abc = np.empty((P, 3 * row), dtype=np.float32)
    off = 0
    for cf in chunks:
        sl = slice(off, off + cf)
        abc[:, 3 * off:3 * off + cf] = a[:, sl]
        abc[:, 3 * off + cf:3 * off + 2 * cf] = b[:, sl]
        abc[:, 3 * off + 2 * cf:3 * off + 3 * cf] = c[:, sl]
        off += cf
    return abc


def kernel(preds1, cams1, preds1_back, preds2, cams2, y, index):
    from concourse.bass_utils import run_bass_kernel_spmd

    idx = int(np.asarray(index))
    preds1 = np.asarray(preds1, dtype=np.float32)
    preds1_back = np.asarray(preds1_back, dtype=np.float32)
    preds2 = np.asarray(preds2, dtype=np.float32)
    cams1 = np.asarray(cams1, dtype=np.float32)
    cams2 = np.asarray(cams2, dtype=np.float32)
    yi = np.asarray(y).astype(np.int64).reshape(B)
    yf = yi.astype(np.float32).reshape(B, 1)

    sel = np.flatnonzero(yi == 1)
    masked = len(sel) <= CAP
    nc = _get_nc(masked)

    in_maps = []
    for k in range(NCORES):
        s = slice(k * BPC, (k + 1) * BPC)
        sm_host = np.concatenate(
            [preds1[idx, s], preds1[1 - idx, s], preds2[idx, s],
             preds1_back[idx, s], yf[s]], axis=1)
        im = {"small": np.ascontiguousarray(
            np.repeat(sm_host, 2, axis=0))}

        if masked:
            sel_k = sel[k * SLOTS:(k + 1) * SLOTS]
            nk = len(sel_k)
            a = np.zeros((SLOTS, HW), dtype=np.float32)
            b = np.zeros((SLOTS, HW), dtype=np.float32)
            c = np.zeros((SLOTS, HW), dtype=np.float32)
            a[:nk] = cams1[idx, sel_k, 1].reshape(nk, HW)
            b[:nk] = cams2[idx, sel_k, 1].reshape(nk, HW)
            c[:nk] = cams1[1 - idx, sel_k, 1].reshape(nk, HW)
            im["abc"] = _interleave(a.reshape(P, QROW), b.reshape(P, QROW),
                                    c.reshape(P, QROW), CHUNKS_MASK)
            sc = np.zeros((SLOTS, 9), dtype=np.float32)
            sc[:nk] = np.concatenate(
                [preds1[idx, sel_k], preds1[1 - idx, sel_k],
                 preds2[idx, sel_k], preds1_back[idx, sel_k],
                 yf[sel_k]], axis=1)
            im["small_cam"] = np.ascontiguousarray(np.repeat(sc, 4, axis=0))
        else:
            a = cams1[idx, s, 1].reshape(P, HALF)
            b = cams2[idx, s, 1].reshape(P, HALF)
            c = cams1[1 - idx, s, 1].reshape(P, HALF)
            im["abc"] = _interleave(a, b, c, CHUNKS_FULL)
        in_maps.append(im)

    trace = bool(int(os.environ.get("KERNEL_TRACE", "0")))
    res = run_bass_kernel_spmd(nc, in_maps, core_ids=list(range(NCORES)),
                               trace=trace)
    kernel.last_exec_time_ns = res.exec_time_ns
    kernel.last_result = res
    total = sum(float(res.results[k]["out"][0, 0]) for k in range(NCORES))
    return np.array(total, dtype=np.float32)


kernel.last_exec_time_ns = None

